# revision 8
# baseline (speedup 1.0000x reference)
"""Bahdanau (additive) attention kernel for Trainium2, 8 NeuronCores.

Math (per batch b):
    q = query @ W1                        (t, u)
    k = value @ W2                        (s, u)
    scores[t, s] = sum_u scale_u * tanh(q[t, u] + k[s, u])
    scores = where(mask[s], scores, -1e9)
    attn = softmax_s(scores)
    context = attn @ value

Sharding: data-parallel over batch — 16 batches, 2 per core, W1/W2/scale
replicated. Each core runs an identical Bass program (SPMD).

Per-core device algorithm (per batch):
  - qT (u, t) and kT (u, s) built with PE transposes + matmuls so `u` lies on
    partitions.
  - Main loop over (t-group of 8, u-tile j): DVE tensor_scalar broadcast-add
    builds tanh args arg[u_j, s] = kT[u_j, s] + qT[u_j, t] (per-partition
    scalar = qT column); ScalarE tanh on fused (128, 8*256) tiles; PE
    contracts over u with the scale column as the 1-column stationary operand
    (LDWEIGHTS is ~free, tanh streams as the moving operand at 2.4 GHz),
    writing score rows into a (1, 8*256) PSUM row-buffer on partition 0.
    PSUM bank rule: start=True clears the whole 2KB bank's has_written bits,
    so only the even row of each bank pair starts its accumulation group.
  - Rows evacuate via one DVE copy, then per-row DMAs scatter them into the
    (t, s) scores tile (engines are partition-locked; DMA is the only
    cross-partition mover).
  - softmax over the free dim without max-subtraction (|scores| <= 22 since
    |tanh|<=1, sum|scale| ~ 22 -> exp stays comfortably in fp32 range).
  - context = attn @ value via PE transposes of attn + 2 matmuls.
"""

import numpy as np
from contextlib import ExitStack

import concourse.bass as bass
from concourse import bacc
import concourse.tile as tile
from concourse import mybir
from concourse.bass import ts
from concourse.bass_utils import run_bass_kernel_spmd
from concourse.masks import make_identity

AF = mybir.ActivationFunctionType
F32 = mybir.dt.float32
U8 = mybir.dt.uint8

B, T, S, D, U = 16, 128, 256, 512, 512
NCORES = 8
BPC = B // NCORES  # batches per core
NJ = U // 128      # u-tiles
ND = D // 128      # d-tiles
NK = S // 128      # s-tiles
G = 8              # t-rows per PSUM row-buffer / tanh group
NG = T // G
NEG = -1e9


def build_bass():
    nc = bacc.Bacc("TRN2", target_bir_lowering=False, debug=False)

    query_d = nc.dram_tensor("query", [BPC, T, D], F32, kind="ExternalInput")
    value_d = nc.dram_tensor("value", [BPC, S, D], F32, kind="ExternalInput")
    mask_d = nc.dram_tensor("mask", [1, BPC, S], U8, kind="ExternalInput")
    w1_d = nc.dram_tensor("W1", [128, ND, U], F32, kind="ExternalInput")   # [p,i,u] = W1[i*128+p, u]
    w2_d = nc.dram_tensor("W2", [128, ND, U], F32, kind="ExternalInput")
    scale_d = nc.dram_tensor("scale", [128, NJ], F32, kind="ExternalInput")  # [p,j] = scale[j*128+p]

    ctx_d = nc.dram_tensor("context", [BPC, T, D], F32, kind="ExternalOutput")
    attn_d = nc.dram_tensor("attn", [BPC, T, S], F32, kind="ExternalOutput")

    with tile.TileContext(nc) as tc, ExitStack() as ctx:
        consts = ctx.enter_context(tc.tile_pool(name="consts", bufs=1))
        pb = ctx.enter_context(tc.tile_pool(name="perbatch", bufs=2))
        stag_pool = ctx.enter_context(tc.tile_pool(name="stag", bufs=2))
        tanh_pool = ctx.enter_context(tc.tile_pool(name="tanh", bufs=6))
        row_pool = ctx.enter_context(tc.tile_pool(name="rowbuf", bufs=2))
        ps_small = ctx.enter_context(tc.tile_pool(name="ps_small", bufs=2, space="PSUM"))
        ps_rows = ctx.enter_context(tc.tile_pool(name="ps_rows", bufs=1, space="PSUM"))
        ps_ctx = ctx.enter_context(tc.tile_pool(name="ps_ctx", bufs=1, space="PSUM"))

        # ---- constants ----
        ident = consts.tile([128, 128], F32)
        make_identity(nc, ident)

        w1_sb = consts.tile([128, ND, U], F32)
        nc.sync.dma_start(w1_sb[:], w1_d[:])
        w2_sb = consts.tile([128, ND, U], F32)
        nc.sync.dma_start(w2_sb[:], w2_d[:])
        scale_sb = consts.tile([128, NJ], F32)
        nc.sync.dma_start(scale_sb[:], scale_d[:])

        for b in range(BPC):
            # ---- load ----
            q_nat = pb.tile([128, D], F32)                      # (t, d)
            nc.sync.dma_start(q_nat[:], query_d[b])
            v_nat = pb.tile([128, NK, D], F32)                  # (s%128, k, d)
            nc.sync.dma_start(v_nat[:], value_d[b].rearrange("(k p) d -> p k d", p=128))

            # mask bias broadcast to (128, S):  (mask-1)*1e9
            mb_u8 = pb.tile([128, S], U8)
            mask_bc = bass.AP(
                tensor=mask_d.ap().tensor, offset=b * S,
                ap=[[0, 128], [1, S]],
            )
            nc.sync.dma_start(mb_u8[:], mask_bc)
            mb_bc = pb.tile([128, S], F32)
            nc.vector.tensor_scalar(
                mb_bc[:], mb_u8[:], 1e9, NEG,
                mybir.AluOpType.mult, mybir.AluOpType.add,
            )

            # ---- transpose query -> qTin (d on partitions) ----
            qTin = pb.tile([128, ND, 128], F32)                 # (d%128, i, t)
            for i in range(ND):
                ps_t = ps_small.tile([128, 128], F32)
                nc.tensor.transpose(ps_t[:], q_nat[:, ts(i, 128)], ident[:])
                nc.vector.tensor_copy(qTin[:, i, :], ps_t[:])

            # ---- transpose value -> vT (d on partitions) ----
            vT = pb.tile([128, ND, S], F32)                     # (d%128, i, s)
            for i in range(ND):
                for k in range(NK):
                    ps_t = ps_small.tile([128, 128], F32)
                    nc.tensor.transpose(ps_t[:], v_nat[:, k, ts(i, 128)], ident[:])
                    nc.vector.tensor_copy(vT[:, i, ts(k, 128)], ps_t[:])

            # ---- qT[u_j, t] = sum_i W1[d_i, u_j].T @ qTin[d_i, t] ----
            qT = pb.tile([128, NJ, 128], F32)                   # (u%128, j, t)
            for j in range(NJ):
                ps_q = ps_small.tile([128, 128], F32, tag="ps_qk")
                for i in range(ND):
                    nc.tensor.matmul(
                        ps_q[:], w1_sb[:, i, ts(j, 128)], qTin[:, i, :],
                        start=(i == 0), stop=(i == ND - 1),
                    )
                nc.vector.tensor_copy(qT[:, j, :], ps_q[:])

            # ---- kT[u_j, s] = sum_i W2[d_i, u_j].T @ vT[d_i, s] ----
            kT = pb.tile([128, NJ, S], F32)                     # (u%128, j, s)
            for j in range(NJ):
                ps_k = ps_small.tile([128, S], F32, tag="ps_qk")
                for i in range(ND):
                    nc.tensor.matmul(
                        ps_k[:], w2_sb[:, i, ts(j, 128)], vT[:, i, :],
                        start=(i == 0), stop=(i == ND - 1),
                    )
                nc.vector.tensor_copy(kT[:, j, :], ps_k[:])

            # ---- main loop: score rows in PSUM row-buffers ----
            scores_sb = pb.tile([128, S], F32)
            for ga in range(NG):
                tanh_tiles = []
                for j in range(NJ):
                    stag = stag_pool.tile([128, G * S], F32)
                    for r in range(G):
                        t = ga * G + r
                        nc.vector.tensor_scalar_add(
                            stag[:, ts(r, S)], kT[:, j, :], qT[:, j, t:t + 1],
                        )
                    tanh_t = tanh_pool.tile([128, G * S], F32)
                    nc.scalar.activation(tanh_t[:], stag[:], AF.Tanh)
                    tanh_tiles.append(tanh_t)
                for half in range(G // 4):
                    prow = ps_rows.tile([1, 4 * S], F32)
                    for rr in range(4):
                        r = half * 4 + rr
                        for j in range(NJ):
                            # bank = 2KB = two (1, S) rows; only the even row
                            # of a bank pair may set start (start clears the
                            # whole bank's has_written bits)
                            nc.tensor.matmul(
                                prow[0:1, ts(rr, S)], scale_sb[:, j:j + 1],
                                tanh_tiles[j][:, ts(r, S)],
                                start=(j == 0 and rr % 2 == 0),
                                stop=(j == NJ - 1 and rr % 2 == 1),
                                skip_group_check=True,
                            )
                    rowbuf = row_pool.tile([1, 4 * S], F32)
                    nc.vector.tensor_copy(rowbuf[:], prow[:])
                    for rr in range(4):
                        t = ga * G + half * 4 + rr
                        nc.sync.dma_start(scores_sb[t:t + 1, :], rowbuf[0:1, ts(rr, S)])

            # ---- mask + softmax over s ----
            masked = pb.tile([128, S], F32)
            nc.vector.tensor_add(masked[:], scores_sb[:], mb_bc[:])
            attn_e = pb.tile([128, S], F32)
            nc.scalar.activation(attn_e[:], masked[:], AF.Exp)
            ssum = pb.tile([128, 1], F32)
            nc.vector.tensor_reduce(ssum[:], attn_e[:], axis=mybir.AxisListType.X,
                                    op=mybir.AluOpType.add)
            rsum = pb.tile([128, 1], F32)
            nc.vector.reciprocal(rsum[:], ssum[:])
            attn_o = pb.tile([128, S], F32)
            nc.vector.tensor_scalar_mul(attn_o[:], attn_e[:], rsum[:])
            nc.sync.dma_start(attn_d[b], attn_o[:])

            # ---- context = attn @ value ----
            attnT = pb.tile([128, NK, 128], F32)                # (s%128, k, t)
            for k in range(NK):
                ps_t = ps_small.tile([128, 128], F32)
                nc.tensor.transpose(ps_t[:], attn_o[:, ts(k, 128)], ident[:])
                nc.vector.tensor_copy(attnT[:, k, :], ps_t[:])
            ps_c = ps_ctx.tile([128, D], F32)
            for k in range(NK):
                nc.tensor.matmul(
                    ps_c[:], attnT[:, k, :], v_nat[:, k, :],
                    start=(k == 0), stop=(k == NK - 1),
                )
            ctx_sb = pb.tile([128, D], F32)
            nc.vector.tensor_copy(ctx_sb[:], ps_c[:])
            nc.sync.dma_start(ctx_d[b], ctx_sb[:])

    nc.compile()
    return nc


_NC_CACHE = None


def _get_nc():
    global _NC_CACHE
    if _NC_CACHE is None:
        _NC_CACHE = build_bass()
    return _NC_CACHE


def _shard_inputs(query, value, mask, W1, W2, scale):
    w1_r = np.ascontiguousarray(
        np.asarray(W1, dtype=np.float32).reshape(ND, 128, U).transpose(1, 0, 2))
    w2_r = np.ascontiguousarray(
        np.asarray(W2, dtype=np.float32).reshape(ND, 128, U).transpose(1, 0, 2))
    scale_r = np.ascontiguousarray(
        np.asarray(scale, dtype=np.float32).reshape(NJ, 128).T)
    in_maps = []
    for c in range(NCORES):
        sl = slice(c * BPC, (c + 1) * BPC)
        in_maps.append({
            "query": np.ascontiguousarray(np.asarray(query[sl], dtype=np.float32)),
            "value": np.ascontiguousarray(np.asarray(value[sl], dtype=np.float32)),
            "mask": np.ascontiguousarray(
                np.asarray(mask[sl]).astype(np.uint8).reshape(1, BPC, S)),
            "W1": w1_r,
            "W2": w2_r,
            "scale": scale_r,
        })
    return in_maps


def run(query, value, mask, W1, W2, scale, **run_kwargs):
    nc = _get_nc()
    in_maps = _shard_inputs(query, value, mask, W1, W2, scale)
    res = run_bass_kernel_spmd(nc, in_maps, core_ids=list(range(NCORES)), **run_kwargs)
    context = np.concatenate([r["context"] for r in res.results], axis=0)
    attn = np.concatenate([r["attn"] for r in res.results], axis=0)
    return (context, attn), res


def kernel(query, value, mask, W1, W2, scale):
    (context, attn), _ = run(query, value, mask, W1, W2, scale)
    return context, attn


# revision 10
# speedup vs baseline: 1.6898x; 1.6898x over previous
"""Bahdanau (additive) attention kernel for Trainium2, 8 NeuronCores.

Math (per batch b):
    q = query @ W1                        (t, u)
    k = value @ W2                        (s, u)
    scores[t, s] = sum_u scale_u * tanh(q[t, u] + k[s, u])
    scores = where(mask[s], scores, -1e9)
    attn = softmax_s(scores)
    context = attn @ value

Sharding: data-parallel over batch — 16 batches, 2 per core, W1/W2/scale
replicated. Each core runs an identical Bass program (SPMD).

Per-core device algorithm (per batch):
  - qT (u, t) and kT (u, s) built with PE transposes + matmuls so `u` lies on
    partitions.
  - Main loop over (t-group of 8, u-tile j): DVE tensor_scalar broadcast-add
    builds tanh args arg[u_j, s] = kT[u_j, s] + qT[u_j, t] (per-partition
    scalar = qT column); ScalarE tanh on fused (128, 8*256) tiles; PE
    contracts over u with the scale column as the 1-column stationary operand
    (LDWEIGHTS is ~free, tanh streams as the moving operand at 2.4 GHz),
    writing score rows into a (1, 8*256) PSUM row-buffer on partition 0.
    PSUM bank rule: start=True clears the whole 2KB bank's has_written bits,
    so only the even row of each bank pair starts its accumulation group.
  - Rows evacuate via one DVE copy, then per-row DMAs scatter them into the
    (t, s) scores tile (engines are partition-locked; DMA is the only
    cross-partition mover).
  - softmax over the free dim without max-subtraction (|scores| <= 22 since
    |tanh|<=1, sum|scale| ~ 22 -> exp stays comfortably in fp32 range).
  - context = attn @ value via PE transposes of attn + 2 matmuls.
"""

import numpy as np
from contextlib import ExitStack

import concourse.bass as bass
from concourse import bacc
import concourse.tile as tile
from concourse import mybir
from concourse.bass import ts
from concourse.bass_utils import run_bass_kernel_spmd
from concourse.masks import make_identity

AF = mybir.ActivationFunctionType
F32 = mybir.dt.float32
F16 = mybir.dt.float16
U8 = mybir.dt.uint8

B, T, S, D, U = 16, 128, 256, 512, 512
NCORES = 8
BPC = B // NCORES  # batches per core
NJ = U // 128      # u-tiles
ND = D // 128      # d-tiles
NK = S // 128      # s-tiles
GA = 16            # t-rows per fused tanh ACT instruction
GP = 8             # t-rows per PSUM row-buffer
NGA = T // GA
NEG = -1e9


def build_bass():
    nc = bacc.Bacc("TRN2", target_bir_lowering=False, debug=False)

    query_d = nc.dram_tensor("query", [BPC, T, D], F32, kind="ExternalInput")
    value_d = nc.dram_tensor("value", [BPC, S, D], F32, kind="ExternalInput")
    mask_d = nc.dram_tensor("mask", [1, BPC, S], U8, kind="ExternalInput")
    w1_d = nc.dram_tensor("W1", [128, ND, U], F32, kind="ExternalInput")   # [p,i,u] = W1[i*128+p, u]
    w2_d = nc.dram_tensor("W2", [128, ND, U], F32, kind="ExternalInput")
    scale_d = nc.dram_tensor("scale", [128, NJ], F32, kind="ExternalInput")  # [p,j] = scale[j*128+p]

    ctx_d = nc.dram_tensor("context", [BPC, T, D], F32, kind="ExternalOutput")
    attn_d = nc.dram_tensor("attn", [BPC, T, S], F32, kind="ExternalOutput")

    with tile.TileContext(nc) as tc, ExitStack() as ctx:
        consts = ctx.enter_context(tc.tile_pool(name="consts", bufs=1))
        pb = ctx.enter_context(tc.tile_pool(name="perbatch", bufs=2))
        stag_pool = ctx.enter_context(tc.tile_pool(name="stag", bufs=2))
        tanh_pool = ctx.enter_context(tc.tile_pool(name="tanh", bufs=6))
        row_pool = ctx.enter_context(tc.tile_pool(name="rowbuf", bufs=2))
        ps_small = ctx.enter_context(tc.tile_pool(name="ps_small", bufs=2, space="PSUM"))
        ps_rows = ctx.enter_context(tc.tile_pool(name="ps_rows", bufs=1, space="PSUM"))
        ps_ctx = ctx.enter_context(tc.tile_pool(name="ps_ctx", bufs=1, space="PSUM"))

        # ---- constants ----
        ident = consts.tile([128, 128], F32)
        make_identity(nc, ident)

        w1_sb = consts.tile([128, ND, U], F32)
        nc.sync.dma_start(w1_sb[:], w1_d[:])
        w2_sb = consts.tile([128, ND, U], F32)
        nc.sync.dma_start(w2_sb[:], w2_d[:])
        scale_sb = consts.tile([128, NJ], F32)
        nc.sync.dma_start(scale_sb[:], scale_d[:])
        scale16 = consts.tile([128, NJ], F16)
        nc.vector.tensor_copy(scale16[:], scale_sb[:])

        for b in range(BPC):
            # ---- load ----
            q_nat = pb.tile([128, D], F32)                      # (t, d)
            nc.sync.dma_start(q_nat[:], query_d[b])
            v_nat = pb.tile([128, NK, D], F32)                  # (s%128, k, d)
            nc.sync.dma_start(v_nat[:], value_d[b].rearrange("(k p) d -> p k d", p=128))

            # mask bias broadcast to (128, S):  (mask-1)*1e9
            mb_u8 = pb.tile([128, S], U8)
            mask_bc = bass.AP(
                tensor=mask_d.ap().tensor, offset=b * S,
                ap=[[0, 128], [1, S]],
            )
            nc.sync.dma_start(mb_u8[:], mask_bc)
            mb_bc = pb.tile([128, S], F32)
            nc.vector.tensor_scalar(
                mb_bc[:], mb_u8[:], 1e9, NEG,
                mybir.AluOpType.mult, mybir.AluOpType.add,
            )

            # ---- transpose query -> qTin (d on partitions) ----
            qTin = pb.tile([128, ND, 128], F32)                 # (d%128, i, t)
            for i in range(ND):
                ps_t = ps_small.tile([128, 128], F32, tag="ps_prep")
                nc.tensor.transpose(ps_t[:], q_nat[:, ts(i, 128)], ident[:])
                nc.vector.tensor_copy(qTin[:, i, :], ps_t[:])

            # ---- transpose value -> vT (d on partitions) ----
            vT = pb.tile([128, ND, S], F32)                     # (d%128, i, s)
            for i in range(ND):
                for k in range(NK):
                    ps_t = ps_small.tile([128, 128], F32, tag="ps_prep")
                    nc.tensor.transpose(ps_t[:], v_nat[:, k, ts(i, 128)], ident[:])
                    nc.vector.tensor_copy(vT[:, i, ts(k, 128)], ps_t[:])

            # ---- qT[u_j, t] = sum_i W1[d_i, u_j].T @ qTin[d_i, t] ----
            qT = pb.tile([128, NJ, 128], F32)  # scalar operand must stay fp32                   # (u%128, j, t)
            for j in range(NJ):
                ps_q = ps_small.tile([128, 128], F32, tag="ps_prep")
                for i in range(ND):
                    nc.tensor.matmul(
                        ps_q[:], w1_sb[:, i, ts(j, 128)], qTin[:, i, :],
                        start=(i == 0), stop=(i == ND - 1),
                    )
                nc.vector.tensor_copy(qT[:, j, :], ps_q[:])

            # ---- kT[u_j, s] = sum_i W2[d_i, u_j].T @ vT[d_i, s] ----
            kT = pb.tile([128, NJ, S], F16)                     # (u%128, j, s)
            for j in range(NJ):
                ps_k = ps_small.tile([128, S], F32, tag="ps_prep")
                for i in range(ND):
                    nc.tensor.matmul(
                        ps_k[:], w2_sb[:, i, ts(j, 128)], vT[:, i, :],
                        start=(i == 0), stop=(i == ND - 1),
                    )
                nc.vector.tensor_copy(kT[:, j, :], ps_k[:])

            # ---- main loop: score rows in PSUM row-buffers ----
            scores_sb = pb.tile([128, S], F32)
            for ga in range(NGA):
                tanh_tiles = []
                for j in range(NJ):
                    stag = stag_pool.tile([128, GA * S], F16)
                    for r in range(GA):
                        t = ga * GA + r
                        nc.vector.tensor_scalar_add(
                            stag[:, ts(r, S)], kT[:, j, :], qT[:, j, t:t + 1],
                        )
                    tanh_t = tanh_pool.tile([128, GA * S], F16)
                    nc.scalar.activation(tanh_t[:], stag[:], AF.Tanh)
                    tanh_tiles.append(tanh_t)
                for h in range(GA // GP):
                    prow = ps_rows.tile([1, GP * S], F32)
                    for j in range(NJ):
                        for m in range(GP // 2):
                            # one matmul covers two rows = one full PSUM bank
                            r = h * GP + m * 2
                            nc.tensor.matmul(
                                prow[0:1, ts(m, 2 * S)], scale16[:, j:j + 1],
                                tanh_tiles[j][:, r * S:(r + 2) * S],
                                start=(j == 0), stop=(j == NJ - 1),
                                skip_group_check=True,
                            )
                    rowbuf = row_pool.tile([1, GP * S], F32)
                    nc.vector.tensor_copy(rowbuf[:], prow[:])
                    for rr in range(GP):
                        t = ga * GA + h * GP + rr
                        nc.sync.dma_start(scores_sb[t:t + 1, :], rowbuf[0:1, ts(rr, S)])

            # ---- mask + softmax over s ----
            masked = pb.tile([128, S], F32)
            nc.vector.tensor_add(masked[:], scores_sb[:], mb_bc[:])
            attn_e = pb.tile([128, S], F32)
            nc.scalar.activation(attn_e[:], masked[:], AF.Exp)
            ssum = pb.tile([128, 1], F32)
            nc.vector.tensor_reduce(ssum[:], attn_e[:], axis=mybir.AxisListType.X,
                                    op=mybir.AluOpType.add)
            rsum = pb.tile([128, 1], F32)
            nc.vector.reciprocal(rsum[:], ssum[:])
            attn_o = pb.tile([128, S], F32)
            nc.vector.tensor_scalar_mul(attn_o[:], attn_e[:], rsum[:])
            nc.sync.dma_start(attn_d[b], attn_o[:])

            # ---- context = attn @ value ----
            attnT = pb.tile([128, NK, 128], F32)                # (s%128, k, t)
            for k in range(NK):
                ps_t = ps_small.tile([128, 128], F32, tag="ps_prep")
                nc.tensor.transpose(ps_t[:], attn_o[:, ts(k, 128)], ident[:])
                nc.vector.tensor_copy(attnT[:, k, :], ps_t[:])
            ps_c = ps_ctx.tile([128, D], F32)
            for k in range(NK):
                nc.tensor.matmul(
                    ps_c[:], attnT[:, k, :], v_nat[:, k, :],
                    start=(k == 0), stop=(k == NK - 1),
                )
            ctx_sb = pb.tile([128, D], F32)
            nc.vector.tensor_copy(ctx_sb[:], ps_c[:])
            nc.sync.dma_start(ctx_d[b], ctx_sb[:])

    nc.compile()
    return nc


_NC_CACHE = None


def _get_nc():
    global _NC_CACHE
    if _NC_CACHE is None:
        _NC_CACHE = build_bass()
    return _NC_CACHE


def _shard_inputs(query, value, mask, W1, W2, scale):
    w1_r = np.ascontiguousarray(
        np.asarray(W1, dtype=np.float32).reshape(ND, 128, U).transpose(1, 0, 2))
    w2_r = np.ascontiguousarray(
        np.asarray(W2, dtype=np.float32).reshape(ND, 128, U).transpose(1, 0, 2))
    scale_r = np.ascontiguousarray(
        np.asarray(scale, dtype=np.float32).reshape(NJ, 128).T)
    in_maps = []
    for c in range(NCORES):
        sl = slice(c * BPC, (c + 1) * BPC)
        in_maps.append({
            "query": np.ascontiguousarray(np.asarray(query[sl], dtype=np.float32)),
            "value": np.ascontiguousarray(np.asarray(value[sl], dtype=np.float32)),
            "mask": np.ascontiguousarray(
                np.asarray(mask[sl]).astype(np.uint8).reshape(1, BPC, S)),
            "W1": w1_r,
            "W2": w2_r,
            "scale": scale_r,
        })
    return in_maps


def run(query, value, mask, W1, W2, scale, **run_kwargs):
    nc = _get_nc()
    in_maps = _shard_inputs(query, value, mask, W1, W2, scale)
    res = run_bass_kernel_spmd(nc, in_maps, core_ids=list(range(NCORES)), **run_kwargs)
    context = np.concatenate([r["context"] for r in res.results], axis=0)
    attn = np.concatenate([r["attn"] for r in res.results], axis=0)
    return (context, attn), res


def kernel(query, value, mask, W1, W2, scale):
    (context, attn), _ = run(query, value, mask, W1, W2, scale)
    return context, attn


# revision 12
# speedup vs baseline: 1.8875x; 1.1170x over previous
"""Bahdanau (additive) attention kernel for Trainium2, 8 NeuronCores.

Math (per batch b):
    q = query @ W1                        (t, u)
    k = value @ W2                        (s, u)
    scores[t, s] = sum_u scale_u * tanh(q[t, u] + k[s, u])
    scores = where(mask[s], scores, -1e9)
    attn = softmax_s(scores)
    context = attn @ value

Sharding: data-parallel over batch — 16 batches, 2 per core, W1/W2/scale
replicated. Each core runs an identical Bass program (SPMD).

Per-core device algorithm (per batch):
  - qT (u, t) and kT (u, s) built with PE transposes + matmuls so `u` lies on
    partitions.
  - Main loop over (t-group of 8, u-tile j): DVE tensor_scalar broadcast-add
    builds tanh args arg[u_j, s] = kT[u_j, s] + qT[u_j, t] (per-partition
    scalar = qT column); ScalarE tanh on fused (128, 8*256) tiles; PE
    contracts over u with the scale column as the 1-column stationary operand
    (LDWEIGHTS is ~free, tanh streams as the moving operand at 2.4 GHz),
    writing score rows into a (1, 8*256) PSUM row-buffer on partition 0.
    PSUM bank rule: start=True clears the whole 2KB bank's has_written bits,
    so only the even row of each bank pair starts its accumulation group.
  - Rows evacuate via one DVE copy, then per-row DMAs scatter them into the
    (t, s) scores tile (engines are partition-locked; DMA is the only
    cross-partition mover).
  - softmax over the free dim without max-subtraction (|scores| <= 22 since
    |tanh|<=1, sum|scale| ~ 22 -> exp stays comfortably in fp32 range).
  - context = attn @ value via PE transposes of attn + 2 matmuls.
"""

import numpy as np
from contextlib import ExitStack

import concourse.bass as bass
from concourse import bacc
import concourse.tile as tile
from concourse import mybir
from concourse.bass import ts
from concourse.bass_utils import run_bass_kernel_spmd
from concourse.masks import make_identity

AF = mybir.ActivationFunctionType
F32 = mybir.dt.float32
F16 = mybir.dt.float16
U8 = mybir.dt.uint8

B, T, S, D, U = 16, 128, 256, 512, 512
NCORES = 8
BPC = B // NCORES  # batches per core
NJ = U // 128      # u-tiles
ND = D // 128      # d-tiles
NK = S // 128      # s-tiles
GA = 16            # t-rows per row-group
DVE_ROWS = 11      # rows of each group whose adds run on DVE (rest: ACT bias)
NGA = T // GA
NEG = -1e9


def build_bass():
    nc = bacc.Bacc("TRN2", target_bir_lowering=False, debug=False)

    query_d = nc.dram_tensor("query", [BPC, T, D], F32, kind="ExternalInput")
    value_d = nc.dram_tensor("value", [BPC, S, D], F32, kind="ExternalInput")
    mask_d = nc.dram_tensor("mask", [1, BPC, S], U8, kind="ExternalInput")
    w1_d = nc.dram_tensor("W1", [128, ND, U], F32, kind="ExternalInput")   # [p,i,u] = W1[i*128+p, u]
    w2_d = nc.dram_tensor("W2", [128, ND, U], F32, kind="ExternalInput")
    scale_d = nc.dram_tensor("scale", [128, NJ, 32], F16, kind="ExternalInput")  # [p,j,m] = scale[j*128+p]

    ctx_d = nc.dram_tensor("context", [BPC, T, D], F32, kind="ExternalOutput")
    attn_d = nc.dram_tensor("attn", [BPC, T, S], F32, kind="ExternalOutput")
    stage_d = nc.dram_tensor("scores_stage", [BPC, T * S], F32)  # internal DRAM bounce

    with tile.TileContext(nc) as tc, ExitStack() as ctx:
        consts = ctx.enter_context(tc.tile_pool(name="consts", bufs=1))
        pb = ctx.enter_context(tc.tile_pool(name="perbatch", bufs=2))
        stag_pool = ctx.enter_context(tc.tile_pool(name="stag", bufs=2))
        tanh_pool = ctx.enter_context(tc.tile_pool(name="tanh", bufs=6))
        row_pool = ctx.enter_context(tc.tile_pool(name="rowbuf", bufs=2))
        ps_small = ctx.enter_context(tc.tile_pool(name="ps_small", bufs=2, space="PSUM"))
        ps_rows = ctx.enter_context(tc.tile_pool(name="ps_rows", bufs=2, space="PSUM"))
        ps_ctx = ctx.enter_context(tc.tile_pool(name="ps_ctx", bufs=1, space="PSUM"))

        # ---- constants ----
        ident = consts.tile([128, 128], F32)
        make_identity(nc, ident)

        w1_sb = consts.tile([128, ND, U], F32)
        nc.sync.dma_start(w1_sb[:], w1_d[:])
        w2_sb = consts.tile([128, ND, U], F32)
        nc.sync.dma_start(w2_sb[:], w2_d[:])
        scale16 = consts.tile([128, NJ, 32], F16)
        nc.sync.dma_start(scale16[:], scale_d[:])
        w1_16 = consts.tile([128, ND, U], F16)
        nc.vector.tensor_copy(w1_16[:], w1_sb[:])
        w2_16 = consts.tile([128, ND, U], F16)
        nc.vector.tensor_copy(w2_16[:], w2_sb[:])

        for b in range(BPC):
            # ---- load ----
            q_nat = pb.tile([128, D], F32)                      # (t, d)
            nc.sync.dma_start(q_nat[:], query_d[b])
            v_nat = pb.tile([128, NK, D], F32)                  # (s%128, k, d)
            nc.sync.dma_start(v_nat[:], value_d[b].rearrange("(k p) d -> p k d", p=128))

            # mask bias broadcast to (128, S):  (mask-1)*1e9
            mb_u8 = pb.tile([128, S], U8)
            mask_bc = bass.AP(
                tensor=mask_d.ap().tensor, offset=b * S,
                ap=[[0, 128], [1, S]],
            )
            nc.sync.dma_start(mb_u8[:], mask_bc)
            mb_bc = pb.tile([128, S], F32)
            nc.vector.tensor_scalar(
                mb_bc[:], mb_u8[:], 1e9, NEG,
                mybir.AluOpType.mult, mybir.AluOpType.add,
            )

            # ---- transpose query -> qTin (d on partitions) ----
            qTin = pb.tile([128, ND, 128], F16)                 # (d%128, i, t)
            for i in range(ND):
                ps_t = ps_small.tile([128, 128], F32, tag="ps_prep")
                nc.tensor.transpose(ps_t[:], q_nat[:, ts(i, 128)], ident[:])
                nc.vector.tensor_copy(qTin[:, i, :], ps_t[:])

            # ---- transpose value -> vT (d on partitions) ----
            vT = pb.tile([128, ND, S], F16)                     # (d%128, i, s)
            for i in range(ND):
                for k in range(NK):
                    ps_t = ps_small.tile([128, 128], F32, tag="ps_prep")
                    nc.tensor.transpose(ps_t[:], v_nat[:, k, ts(i, 128)], ident[:])
                    nc.vector.tensor_copy(vT[:, i, ts(k, 128)], ps_t[:])

            # ---- qT[u_j, t] = sum_i W1[d_i, u_j].T @ qTin[d_i, t] ----
            qT = pb.tile([128, NJ, 128], F32)  # scalar operand must stay fp32                   # (u%128, j, t)
            for j in range(NJ):
                ps_q = ps_small.tile([128, 128], F32, tag="ps_prep")
                for i in range(ND):
                    nc.tensor.matmul(
                        ps_q[:], w1_16[:, i, ts(j, 128)], qTin[:, i, :],
                        start=(i == 0), stop=(i == ND - 1),
                    )
                nc.vector.tensor_copy(qT[:, j, :], ps_q[:])

            # ---- kT[u_j, s] = sum_i W2[d_i, u_j].T @ vT[d_i, s] ----
            kT = pb.tile([128, NJ, S], F16)                     # (u%128, j, s)
            for j in range(NJ):
                ps_k = ps_small.tile([128, S], F32, tag="ps_prep")
                for i in range(ND):
                    nc.tensor.matmul(
                        ps_k[:], w2_16[:, i, ts(j, 128)], vT[:, i, :],
                        start=(i == 0), stop=(i == ND - 1),
                    )
                nc.vector.tensor_copy(kT[:, j, :], ps_k[:])

            # ---- main loop: score rows in PSUM row-buffers ----
            # Per GA=16-row group: DVE builds tanh args for rows [0, DVE_ROWS)
            # via tensor_scalar broadcast-add; ScalarE handles the remaining
            # rows fused into the tanh via its per-partition bias operand.
            # Row pair p (rows 2p, 2p+1) -> PE col-strip c=p//2 (tile_position
            # (0,32c), M=32 replicated scale so a whole strip fills), PSUM
            # half h=p%2. Strip c thus holds rows [4c, 4c+4): one wide DVE
            # copy evacuates all 16 rows, one DMA per strip scatters 4 rows.
            for ga in range(NGA):
                tanh_tiles = []
                for j in range(NJ):
                    stag = stag_pool.tile([128, DVE_ROWS * S], F16)
                    for r in range(DVE_ROWS):
                        t = ga * GA + r
                        nc.vector.tensor_scalar_add(
                            stag[:, ts(r, S)], kT[:, j, :], qT[:, j, t:t + 1],
                        )
                    tanh_t = tanh_pool.tile([128, GA * S], F16)
                    nc.scalar.activation(
                        tanh_t[:, 0:DVE_ROWS * S], stag[:], AF.Tanh)
                    for r in range(DVE_ROWS, GA):
                        t = ga * GA + r
                        nc.scalar.activation(
                            tanh_t[:, ts(r, S)], kT[:, j, :], AF.Tanh,
                            bias=qT[:, j, t:t + 1],
                        )
                    tanh_tiles.append(tanh_t)
                prow = ps_rows.tile([128, 4 * S], F32)
                for j in range(NJ):
                    for p in range(GA // 2):
                        c, h = p // 2, p % 2
                        r = 2 * p
                        nc.tensor.matmul(
                            prow[32 * c:32 * c + 32, ts(h, 2 * S)],
                            scale16[:, j, :], tanh_tiles[j][:, r * S:(r + 2) * S],
                            start=(j == 0), stop=(j == NJ - 1),
                            tile_position=(0, 32 * c),
                            skip_group_check=True,
                        )
                rowbuf = row_pool.tile([128, 4 * S], F32)
                nc.vector.tensor_copy(rowbuf[:], prow[:])
                for c in range(4):
                    base = (ga * GA + 4 * c) * S
                    nc.sync.dma_start(
                        stage_d[b, base:base + 4 * S].rearrange("(o x) -> o x", o=1),
                        rowbuf[32 * c:32 * c + 1, :],
                    )

            # gather the staged scores back as a (t, s) tile
            scores_sb = pb.tile([128, S], F32)
            nc.sync.dma_start(scores_sb[:], stage_d[b].rearrange("(t s) -> t s", s=S))

            # ---- mask + softmax over s ----
            masked = pb.tile([128, S], F32)
            nc.vector.tensor_add(masked[:], scores_sb[:], mb_bc[:])
            attn_e = pb.tile([128, S], F32)
            nc.scalar.activation(attn_e[:], masked[:], AF.Exp)
            ssum = pb.tile([128, 1], F32)
            nc.vector.tensor_reduce(ssum[:], attn_e[:], axis=mybir.AxisListType.X,
                                    op=mybir.AluOpType.add)
            rsum = pb.tile([128, 1], F32)
            nc.vector.reciprocal(rsum[:], ssum[:])
            attn_o = pb.tile([128, S], F32)
            nc.vector.tensor_scalar_mul(attn_o[:], attn_e[:], rsum[:])
            nc.sync.dma_start(attn_d[b], attn_o[:])

            # ---- context = attn @ value ----
            attnT = pb.tile([128, NK, 128], F32)                # (s%128, k, t)
            for k in range(NK):
                ps_t = ps_small.tile([128, 128], F32, tag="ps_prep")
                nc.tensor.transpose(ps_t[:], attn_o[:, ts(k, 128)], ident[:])
                nc.vector.tensor_copy(attnT[:, k, :], ps_t[:])
            ps_c = ps_ctx.tile([128, D], F32)
            for k in range(NK):
                nc.tensor.matmul(
                    ps_c[:], attnT[:, k, :], v_nat[:, k, :],
                    start=(k == 0), stop=(k == NK - 1),
                )
            ctx_sb = pb.tile([128, D], F32)
            nc.vector.tensor_copy(ctx_sb[:], ps_c[:])
            nc.sync.dma_start(ctx_d[b], ctx_sb[:])

    nc.compile()
    return nc


_NC_CACHE = None


def _get_nc():
    global _NC_CACHE
    if _NC_CACHE is None:
        _NC_CACHE = build_bass()
    return _NC_CACHE


def _shard_inputs(query, value, mask, W1, W2, scale):
    w1_r = np.ascontiguousarray(
        np.asarray(W1, dtype=np.float32).reshape(ND, 128, U).transpose(1, 0, 2))
    w2_r = np.ascontiguousarray(
        np.asarray(W2, dtype=np.float32).reshape(ND, 128, U).transpose(1, 0, 2))
    scale_r = np.ascontiguousarray(np.broadcast_to(
        np.asarray(scale, dtype=np.float32).reshape(NJ, 128).T.astype(np.float16)[:, :, None],
        (128, NJ, 32)))
    in_maps = []
    for c in range(NCORES):
        sl = slice(c * BPC, (c + 1) * BPC)
        in_maps.append({
            "query": np.ascontiguousarray(np.asarray(query[sl], dtype=np.float32)),
            "value": np.ascontiguousarray(np.asarray(value[sl], dtype=np.float32)),
            "mask": np.ascontiguousarray(
                np.asarray(mask[sl]).astype(np.uint8).reshape(1, BPC, S)),
            "W1": w1_r,
            "W2": w2_r,
            "scale": scale_r,
        })
    return in_maps


def run(query, value, mask, W1, W2, scale, **run_kwargs):
    nc = _get_nc()
    in_maps = _shard_inputs(query, value, mask, W1, W2, scale)
    res = run_bass_kernel_spmd(nc, in_maps, core_ids=list(range(NCORES)), **run_kwargs)
    context = np.concatenate([r["context"] for r in res.results], axis=0)
    attn = np.concatenate([r["attn"] for r in res.results], axis=0)
    return (context, attn), res


def kernel(query, value, mask, W1, W2, scale):
    (context, attn), _ = run(query, value, mask, W1, W2, scale)
    return context, attn


# revision 13
# speedup vs baseline: 2.0172x; 1.0687x over previous
"""Bahdanau (additive) attention kernel for Trainium2, 8 NeuronCores.

Math (per batch b):
    q = query @ W1                        (t, u)
    k = value @ W2                        (s, u)
    scores[t, s] = sum_u scale_u * tanh(q[t, u] + k[s, u])
    scores = where(mask[s], scores, -1e9)
    attn = softmax_s(scores)
    context = attn @ value

Sharding: data-parallel over batch — 16 batches, 2 per core, W1/W2/scale
replicated. Each core runs an identical Bass program (SPMD).

Per-core device algorithm (per batch):
  - qT (u, t) and kT (u, s) built with PE transposes + matmuls so `u` lies on
    partitions.
  - Main loop over (t-group of 8, u-tile j): DVE tensor_scalar broadcast-add
    builds tanh args arg[u_j, s] = kT[u_j, s] + qT[u_j, t] (per-partition
    scalar = qT column); ScalarE tanh on fused (128, 8*256) tiles; PE
    contracts over u with the scale column as the 1-column stationary operand
    (LDWEIGHTS is ~free, tanh streams as the moving operand at 2.4 GHz),
    writing score rows into a (1, 8*256) PSUM row-buffer on partition 0.
    PSUM bank rule: start=True clears the whole 2KB bank's has_written bits,
    so only the even row of each bank pair starts its accumulation group.
  - Rows evacuate via one DVE copy, then per-row DMAs scatter them into the
    (t, s) scores tile (engines are partition-locked; DMA is the only
    cross-partition mover).
  - softmax over the free dim without max-subtraction (|scores| <= 22 since
    |tanh|<=1, sum|scale| ~ 22 -> exp stays comfortably in fp32 range).
  - context = attn @ value via PE transposes of attn + 2 matmuls.
"""

import numpy as np
from contextlib import ExitStack

import concourse.bass as bass
from concourse import bacc
import concourse.tile as tile
from concourse import mybir
from concourse.bass import ts
from concourse.bass_utils import run_bass_kernel_spmd
from concourse.masks import make_identity

AF = mybir.ActivationFunctionType
F32 = mybir.dt.float32
F16 = mybir.dt.float16
U8 = mybir.dt.uint8

B, T, S, D, U = 16, 128, 256, 512, 512
NCORES = 8
BPC = B // NCORES  # batches per core
NJ = U // 128      # u-tiles
ND = D // 128      # d-tiles
NK = S // 128      # s-tiles
GA = 16            # t-rows per row-group
DVE_ROWS = 13      # rows of each group whose adds run on DVE (rest: ACT bias)
NGA = T // GA
NEG = -1e9


def build_bass():
    nc = bacc.Bacc("TRN2", target_bir_lowering=False, debug=False)

    query_d = nc.dram_tensor("query", [BPC, T, D], F32, kind="ExternalInput")
    value_d = nc.dram_tensor("value", [BPC, S, D], F32, kind="ExternalInput")
    mask_d = nc.dram_tensor("mask", [1, BPC, S], U8, kind="ExternalInput")
    w1_d = nc.dram_tensor("W1", [128, ND, U], F32, kind="ExternalInput")   # [p,i,u] = W1[i*128+p, u]
    w2_d = nc.dram_tensor("W2", [128, ND, U], F32, kind="ExternalInput")
    scale_d = nc.dram_tensor("scale", [128, NJ, 32], F16, kind="ExternalInput")  # [p,j,m] = scale[j*128+p]

    ctx_d = nc.dram_tensor("context", [BPC, T, D], F32, kind="ExternalOutput")
    attn_d = nc.dram_tensor("attn", [BPC, T, S], F32, kind="ExternalOutput")
    stage_d = nc.dram_tensor("scores_stage", [BPC, T * S], F32)  # internal DRAM bounce

    with tile.TileContext(nc) as tc, ExitStack() as ctx:
        consts = ctx.enter_context(tc.tile_pool(name="consts", bufs=1))
        pb = ctx.enter_context(tc.tile_pool(name="perbatch", bufs=2))
        stag_pool = ctx.enter_context(tc.tile_pool(name="stag", bufs=2))
        tanh_pool = ctx.enter_context(tc.tile_pool(name="tanh", bufs=6))
        row_pool = ctx.enter_context(tc.tile_pool(name="rowbuf", bufs=2))
        ps_small = ctx.enter_context(tc.tile_pool(name="ps_small", bufs=2, space="PSUM"))
        ps_rows = ctx.enter_context(tc.tile_pool(name="ps_rows", bufs=2, space="PSUM"))
        ps_ctx = ctx.enter_context(tc.tile_pool(name="ps_ctx", bufs=1, space="PSUM"))

        # ---- constants ----
        ident = consts.tile([128, 128], F32)
        make_identity(nc, ident)

        w1_sb = consts.tile([128, ND, U], F32)
        nc.sync.dma_start(w1_sb[:], w1_d[:])
        w2_sb = consts.tile([128, ND, U], F32)
        nc.sync.dma_start(w2_sb[:], w2_d[:])
        scale16 = consts.tile([128, NJ, 32], F16)
        nc.sync.dma_start(scale16[:], scale_d[:])
        w1_16 = consts.tile([128, ND, U], F16)
        nc.vector.tensor_copy(w1_16[:], w1_sb[:])
        w2_16 = consts.tile([128, ND, U], F16)
        nc.vector.tensor_copy(w2_16[:], w2_sb[:])

        for b in range(BPC):
            # ---- load ----
            q_nat = pb.tile([128, D], F32)                      # (t, d)
            nc.sync.dma_start(q_nat[:], query_d[b])
            v_nat = pb.tile([128, NK, D], F32)                  # (s%128, k, d)
            nc.sync.dma_start(v_nat[:], value_d[b].rearrange("(k p) d -> p k d", p=128))

            # mask bias broadcast to (128, S):  (mask-1)*1e9
            mb_u8 = pb.tile([128, S], U8)
            mask_bc = bass.AP(
                tensor=mask_d.ap().tensor, offset=b * S,
                ap=[[0, 128], [1, S]],
            )
            nc.sync.dma_start(mb_u8[:], mask_bc)
            mb_bc = pb.tile([128, S], F32)
            nc.vector.tensor_scalar(
                mb_bc[:], mb_u8[:], 1e9, NEG,
                mybir.AluOpType.mult, mybir.AluOpType.add,
            )

            # ---- transpose query -> qTin (d on partitions) ----
            qTin = pb.tile([128, ND, 128], F16)                 # (d%128, i, t)
            for i in range(ND):
                ps_t = ps_small.tile([128, 128], F32, tag="ps_prep")
                nc.tensor.transpose(ps_t[:], q_nat[:, ts(i, 128)], ident[:])
                nc.scalar.copy(qTin[:, i, :], ps_t[:])

            # ---- transpose value -> vT (d on partitions) ----
            vT = pb.tile([128, ND, S], F16)                     # (d%128, i, s)
            for i in range(ND):
                for k in range(NK):
                    ps_t = ps_small.tile([128, 128], F32, tag="ps_prep")
                    nc.tensor.transpose(ps_t[:], v_nat[:, k, ts(i, 128)], ident[:])
                    nc.scalar.copy(vT[:, i, ts(k, 128)], ps_t[:])

            # ---- qT[u_j, t] = sum_i W1[d_i, u_j].T @ qTin[d_i, t] ----
            qT = pb.tile([128, NJ, 128], F32)  # scalar operand must stay fp32                   # (u%128, j, t)
            for j in range(NJ):
                ps_q = ps_small.tile([128, 128], F32, tag="ps_prep")
                for i in range(ND):
                    nc.tensor.matmul(
                        ps_q[:], w1_16[:, i, ts(j, 128)], qTin[:, i, :],
                        start=(i == 0), stop=(i == ND - 1),
                    )
                nc.vector.tensor_copy(qT[:, j, :], ps_q[:])

            # ---- kT[u_j, s] = sum_i W2[d_i, u_j].T @ vT[d_i, s] ----
            kT = pb.tile([128, NJ, S], F16)                     # (u%128, j, s)
            for j in range(NJ):
                ps_k = ps_small.tile([128, S], F32, tag="ps_prep")
                for i in range(ND):
                    nc.tensor.matmul(
                        ps_k[:], w2_16[:, i, ts(j, 128)], vT[:, i, :],
                        start=(i == 0), stop=(i == ND - 1),
                    )
                nc.vector.tensor_copy(kT[:, j, :], ps_k[:])

            # ---- main loop: score rows in PSUM row-buffers ----
            # Per GA=16-row group: DVE builds tanh args for rows [0, DVE_ROWS)
            # via tensor_scalar broadcast-add; ScalarE handles the remaining
            # rows fused into the tanh via its per-partition bias operand.
            # Row pair p (rows 2p, 2p+1) -> PE col-strip c=p//2 (tile_position
            # (0,32c), M=32 replicated scale so a whole strip fills), PSUM
            # half h=p%2. Strip c thus holds rows [4c, 4c+4): one wide DVE
            # copy evacuates all 16 rows, one DMA per strip scatters 4 rows.
            for ga in range(NGA):
                tanh_tiles = []
                for j in range(NJ):
                    stag = stag_pool.tile([128, DVE_ROWS * S], F16)
                    for r in range(DVE_ROWS):
                        t = ga * GA + r
                        nc.vector.tensor_scalar_add(
                            stag[:, ts(r, S)], kT[:, j, :], qT[:, j, t:t + 1],
                        )
                    tanh_t = tanh_pool.tile([128, GA * S], F16)
                    nc.scalar.activation(
                        tanh_t[:, 0:DVE_ROWS * S], stag[:], AF.Tanh)
                    for r in range(DVE_ROWS, GA):
                        t = ga * GA + r
                        nc.scalar.activation(
                            tanh_t[:, ts(r, S)], kT[:, j, :], AF.Tanh,
                            bias=qT[:, j, t:t + 1],
                        )
                    tanh_tiles.append(tanh_t)
                prow = ps_rows.tile([128, 4 * S], F32)
                for j in range(NJ):
                    for p in range(GA // 2):
                        c, h = p // 2, p % 2
                        r = 2 * p
                        nc.tensor.matmul(
                            prow[32 * c:32 * c + 32, ts(h, 2 * S)],
                            scale16[:, j, :], tanh_tiles[j][:, r * S:(r + 2) * S],
                            start=(j == 0), stop=(j == NJ - 1),
                            tile_position=(0, 32 * c),
                            skip_group_check=True,
                        )
                rowbuf = row_pool.tile([128, 4 * S], F32)
                nc.vector.tensor_copy(rowbuf[:], prow[:])
                for c in range(4):
                    base = (ga * GA + 4 * c) * S
                    nc.sync.dma_start(
                        stage_d[b, base:base + 4 * S].rearrange("(o x) -> o x", o=1),
                        rowbuf[32 * c:32 * c + 1, :],
                    )

            # gather the staged scores back as a (t, s) tile
            scores_sb = pb.tile([128, S], F32)
            nc.sync.dma_start(scores_sb[:], stage_d[b].rearrange("(t s) -> t s", s=S))

            # ---- mask + softmax over s ----
            masked = pb.tile([128, S], F32)
            nc.vector.tensor_add(masked[:], scores_sb[:], mb_bc[:])
            attn_e = pb.tile([128, S], F32)
            nc.scalar.activation(attn_e[:], masked[:], AF.Exp)
            ssum = pb.tile([128, 1], F32)
            nc.vector.tensor_reduce(ssum[:], attn_e[:], axis=mybir.AxisListType.X,
                                    op=mybir.AluOpType.add)
            rsum = pb.tile([128, 1], F32)
            nc.vector.reciprocal(rsum[:], ssum[:])
            attn_o = pb.tile([128, S], F32)
            nc.vector.tensor_scalar_mul(attn_o[:], attn_e[:], rsum[:])
            nc.sync.dma_start(attn_d[b], attn_o[:])

            # ---- context = attn @ value ----
            attnT = pb.tile([128, NK, 128], F32)                # (s%128, k, t)
            for k in range(NK):
                ps_t = ps_small.tile([128, 128], F32, tag="ps_prep")
                nc.tensor.transpose(ps_t[:], attn_o[:, ts(k, 128)], ident[:])
                nc.vector.tensor_copy(attnT[:, k, :], ps_t[:])
            ps_c = ps_ctx.tile([128, D], F32)
            for k in range(NK):
                nc.tensor.matmul(
                    ps_c[:], attnT[:, k, :], v_nat[:, k, :],
                    start=(k == 0), stop=(k == NK - 1),
                )
            ctx_sb = pb.tile([128, D], F32)
            nc.vector.tensor_copy(ctx_sb[:], ps_c[:])
            nc.sync.dma_start(ctx_d[b], ctx_sb[:])

    nc.compile()
    return nc


_NC_CACHE = None


def _get_nc():
    global _NC_CACHE
    if _NC_CACHE is None:
        _NC_CACHE = build_bass()
    return _NC_CACHE


def _shard_inputs(query, value, mask, W1, W2, scale):
    w1_r = np.ascontiguousarray(
        np.asarray(W1, dtype=np.float32).reshape(ND, 128, U).transpose(1, 0, 2))
    w2_r = np.ascontiguousarray(
        np.asarray(W2, dtype=np.float32).reshape(ND, 128, U).transpose(1, 0, 2))
    scale_r = np.ascontiguousarray(np.broadcast_to(
        np.asarray(scale, dtype=np.float32).reshape(NJ, 128).T.astype(np.float16)[:, :, None],
        (128, NJ, 32)))
    in_maps = []
    for c in range(NCORES):
        sl = slice(c * BPC, (c + 1) * BPC)
        in_maps.append({
            "query": np.ascontiguousarray(np.asarray(query[sl], dtype=np.float32)),
            "value": np.ascontiguousarray(np.asarray(value[sl], dtype=np.float32)),
            "mask": np.ascontiguousarray(
                np.asarray(mask[sl]).astype(np.uint8).reshape(1, BPC, S)),
            "W1": w1_r,
            "W2": w2_r,
            "scale": scale_r,
        })
    return in_maps


def run(query, value, mask, W1, W2, scale, **run_kwargs):
    nc = _get_nc()
    in_maps = _shard_inputs(query, value, mask, W1, W2, scale)
    res = run_bass_kernel_spmd(nc, in_maps, core_ids=list(range(NCORES)), **run_kwargs)
    context = np.concatenate([r["context"] for r in res.results], axis=0)
    attn = np.concatenate([r["attn"] for r in res.results], axis=0)
    return (context, attn), res


def kernel(query, value, mask, W1, W2, scale):
    (context, attn), _ = run(query, value, mask, W1, W2, scale)
    return context, attn


# revision 14
# speedup vs baseline: 2.1747x; 1.0781x over previous
"""Bahdanau (additive) attention kernel for Trainium2, 8 NeuronCores.

Math (per batch b):
    q = query @ W1                        (t, u)
    k = value @ W2                        (s, u)
    scores[t, s] = sum_u scale_u * tanh(q[t, u] + k[s, u])
    scores = where(mask[s], scores, -1e9)
    attn = softmax_s(scores)
    context = attn @ value

Sharding: data-parallel over batch — 16 batches, 2 per core, W1/W2/scale
replicated. Each core runs an identical Bass program (SPMD).

Per-core device algorithm (per batch):
  - qT (u, t) and kT (u, s) built with PE transposes + matmuls so `u` lies on
    partitions.
  - Main loop over (t-group of 8, u-tile j): DVE tensor_scalar broadcast-add
    builds tanh args arg[u_j, s] = kT[u_j, s] + qT[u_j, t] (per-partition
    scalar = qT column); ScalarE tanh on fused (128, 8*256) tiles; PE
    contracts over u with the scale column as the 1-column stationary operand
    (LDWEIGHTS is ~free, tanh streams as the moving operand at 2.4 GHz),
    writing score rows into a (1, 8*256) PSUM row-buffer on partition 0.
    PSUM bank rule: start=True clears the whole 2KB bank's has_written bits,
    so only the even row of each bank pair starts its accumulation group.
  - Rows evacuate via one DVE copy, then per-row DMAs scatter them into the
    (t, s) scores tile (engines are partition-locked; DMA is the only
    cross-partition mover).
  - softmax over the free dim without max-subtraction (|scores| <= 22 since
    |tanh|<=1, sum|scale| ~ 22 -> exp stays comfortably in fp32 range).
  - context = attn @ value via PE transposes of attn + 2 matmuls.
"""

import numpy as np
from contextlib import ExitStack

import concourse.bass as bass
from concourse import bacc
import concourse.tile as tile
from concourse import mybir
from concourse.bass import ts
from concourse.bass_utils import run_bass_kernel_spmd
from concourse.masks import make_identity

AF = mybir.ActivationFunctionType
F32 = mybir.dt.float32
F16 = mybir.dt.float16
U8 = mybir.dt.uint8

B, T, S, D, U = 16, 128, 256, 512, 512
NCORES = 8
BPC = B // NCORES  # batches per core
NJ = U // 128      # u-tiles
ND = D // 128      # d-tiles
NK = S // 128      # s-tiles
GA = 16            # t-rows per row-group
DVE_ROWS = 13      # rows of each group whose adds run on DVE (rest: ACT bias)
NGA = T // GA
NEG = -1e9


def build_bass():
    nc = bacc.Bacc("TRN2", target_bir_lowering=False, debug=False)

    query_d = nc.dram_tensor("query", [BPC, T, D], F32, kind="ExternalInput")
    value_d = nc.dram_tensor("value", [BPC, S, D], F32, kind="ExternalInput")
    mask_d = nc.dram_tensor("mask", [1, BPC, S], U8, kind="ExternalInput")
    w1_d = nc.dram_tensor("W1", [128, ND, U], F32, kind="ExternalInput")   # [p,i,u] = W1[i*128+p, u]
    w2_d = nc.dram_tensor("W2", [128, ND, U], F32, kind="ExternalInput")
    scale_d = nc.dram_tensor("scale", [128, NJ, 32], F16, kind="ExternalInput")  # [p,j,m] = scale[j*128+p]

    ctx_d = nc.dram_tensor("context", [BPC, T, D], F32, kind="ExternalOutput")
    attn_d = nc.dram_tensor("attn", [BPC, T, S], F32, kind="ExternalOutput")
    stage_d = nc.dram_tensor("scores_stage", [BPC, T * S], F32)  # internal DRAM bounce

    with tile.TileContext(nc) as tc, ExitStack() as ctx:
        consts = ctx.enter_context(tc.tile_pool(name="consts", bufs=1))
        pb = ctx.enter_context(tc.tile_pool(name="perbatch", bufs=2))
        stag_pool = ctx.enter_context(tc.tile_pool(name="stag", bufs=2))
        tanh_pool = ctx.enter_context(tc.tile_pool(name="tanh", bufs=6))
        row_pool = ctx.enter_context(tc.tile_pool(name="rowbuf", bufs=2))
        ps_small = ctx.enter_context(tc.tile_pool(name="ps_small", bufs=2, space="PSUM"))
        ps_rows = ctx.enter_context(tc.tile_pool(name="ps_rows", bufs=2, space="PSUM"))
        ps_ctx = ctx.enter_context(tc.tile_pool(name="ps_ctx", bufs=1, space="PSUM"))

        # ---- constants ----
        ident = consts.tile([128, 128], F32)
        make_identity(nc, ident)

        w1_sb = consts.tile([128, ND, U], F32)
        nc.sync.dma_start(w1_sb[:], w1_d[:])
        w2_sb = consts.tile([128, ND, U], F32)
        nc.sync.dma_start(w2_sb[:], w2_d[:])
        scale16 = consts.tile([128, NJ, 32], F16)
        nc.sync.dma_start(scale16[:], scale_d[:])
        w1_16 = consts.tile([128, ND, U], F16)
        nc.vector.tensor_copy(w1_16[:], w1_sb[:])
        w2_16 = consts.tile([128, ND, U], F16)
        nc.vector.tensor_copy(w2_16[:], w2_sb[:])

        qTs, kTs, v_nats, mb_bcs = [], [], [], []
        for b in range(BPC):
            # ---- load ----
            q_nat = pb.tile([128, D], F32)                      # (t, d)
            nc.sync.dma_start(q_nat[:], query_d[b])
            v_nat = pb.tile([128, NK, D], F32)                  # (s%128, k, d)
            nc.sync.dma_start(v_nat[:], value_d[b].rearrange("(k p) d -> p k d", p=128))

            # mask bias broadcast to (128, S):  (mask-1)*1e9
            mb_u8 = pb.tile([128, S], U8)
            mask_bc = bass.AP(
                tensor=mask_d.ap().tensor, offset=b * S,
                ap=[[0, 128], [1, S]],
            )
            nc.sync.dma_start(mb_u8[:], mask_bc)
            mb_bc = pb.tile([128, S], F32)
            nc.vector.tensor_scalar(
                mb_bc[:], mb_u8[:], 1e9, NEG,
                mybir.AluOpType.mult, mybir.AluOpType.add,
            )

            # ---- transpose query -> qTin (d on partitions) ----
            qTin = pb.tile([128, ND, 128], F16)                 # (d%128, i, t)
            for i in range(ND):
                ps_t = ps_small.tile([128, 128], F32, tag="ps_prep")
                nc.tensor.transpose(ps_t[:], q_nat[:, ts(i, 128)], ident[:])
                nc.scalar.copy(qTin[:, i, :], ps_t[:])

            # ---- transpose value -> vT (d on partitions) ----
            vT = pb.tile([128, ND, S], F16)                     # (d%128, i, s)
            for i in range(ND):
                for k in range(NK):
                    ps_t = ps_small.tile([128, 128], F32, tag="ps_prep")
                    nc.tensor.transpose(ps_t[:], v_nat[:, k, ts(i, 128)], ident[:])
                    nc.scalar.copy(vT[:, i, ts(k, 128)], ps_t[:])

            # ---- qT[u_j, t] = sum_i W1[d_i, u_j].T @ qTin[d_i, t] ----
            qT = pb.tile([128, NJ, 128], F32)   # fp32: DVE scalar operand
            for j in range(NJ):
                ps_q = ps_small.tile([128, 128], F32, tag="ps_prep")
                for i in range(ND):
                    nc.tensor.matmul(
                        ps_q[:], w1_16[:, i, ts(j, 128)], qTin[:, i, :],
                        start=(i == 0), stop=(i == ND - 1),
                    )
                nc.vector.tensor_copy(qT[:, j, :], ps_q[:])

            # ---- kT[u_j, s] = sum_i W2[d_i, u_j].T @ vT[d_i, s] ----
            kT = pb.tile([128, NJ, S], F16)                     # (u%128, j, s)
            for j in range(NJ):
                ps_k = ps_small.tile([128, S], F32, tag="ps_prep")
                for i in range(ND):
                    nc.tensor.matmul(
                        ps_k[:], w2_16[:, i, ts(j, 128)], vT[:, i, :],
                        start=(i == 0), stop=(i == ND - 1),
                    )
                nc.vector.tensor_copy(kT[:, j, :], ps_k[:])
            qTs.append(qT); kTs.append(kT); v_nats.append(v_nat); mb_bcs.append(mb_bc)

        # ---- main loops, batches interleaved per row-group ----
        # Per GA=16-row group: DVE builds tanh args for the first DVE_ROWS[j]
        # rows via tensor_scalar broadcast-add; ScalarE handles the remaining
        # rows fused into its tanh via the per-partition bias operand.
        # Row pair p (rows 2p, 2p+1) -> PE col-strip c=p//2 (tile_position
        # (0,32c), M=32 replicated scale so a whole strip fills), PSUM half
        # h=p%2. Strip c holds rows [4c, 4c+4): one wide DVE copy evacuates
        # 16 rows, one DMA per strip stores 4 contiguous rows to a DRAM
        # staging buffer (engines cannot scatter across partitions).
        for ga in range(NGA):
            for b in range(BPC):
                qT, kT = qTs[b], kTs[b]
                tanh_tiles = []
                for j in range(NJ):
                    dr = DVE_ROWS + (1 if j >= 2 else 0)
                    stag = stag_pool.tile([128, (DVE_ROWS + 1) * S], F16)
                    for r in range(dr):
                        t = ga * GA + r
                        nc.vector.tensor_scalar_add(
                            stag[:, ts(r, S)], kT[:, j, :], qT[:, j, t:t + 1],
                        )
                    tanh_t = tanh_pool.tile([128, GA * S], F16)
                    nc.scalar.activation(
                        tanh_t[:, 0:dr * S], stag[:, 0:dr * S], AF.Tanh)
                    for r in range(dr, GA):
                        t = ga * GA + r
                        nc.scalar.activation(
                            tanh_t[:, ts(r, S)], kT[:, j, :], AF.Tanh,
                            bias=qT[:, j, t:t + 1],
                        )
                    tanh_tiles.append(tanh_t)
                prow = ps_rows.tile([128, 4 * S], F32)
                for j in range(NJ):
                    for p in range(GA // 2):
                        c, h = p // 2, p % 2
                        r = 2 * p
                        nc.tensor.matmul(
                            prow[32 * c:32 * c + 32, ts(h, 2 * S)],
                            scale16[:, j, :], tanh_tiles[j][:, r * S:(r + 2) * S],
                            start=(j == 0), stop=(j == NJ - 1),
                            tile_position=(0, 32 * c),
                            skip_group_check=True,
                        )
                rowbuf = row_pool.tile([128, 4 * S], F32)
                nc.vector.tensor_copy(rowbuf[:], prow[:])
                for c in range(4):
                    base = (ga * GA + 4 * c) * S
                    nc.sync.dma_start(
                        stage_d[b, base:base + 4 * S].rearrange("(o x) -> o x", o=1),
                        rowbuf[32 * c:32 * c + 1, :],
                    )

        for b in range(BPC):
            # gather the staged scores back as a (t, s) tile
            scores_sb = pb.tile([128, S], F32)
            nc.sync.dma_start(scores_sb[:], stage_d[b].rearrange("(t s) -> t s", s=S))

            # ---- mask + softmax over s ----
            masked = pb.tile([128, S], F32)
            nc.vector.tensor_add(masked[:], scores_sb[:], mb_bcs[b][:])
            attn_e = pb.tile([128, S], F32)
            nc.scalar.activation(attn_e[:], masked[:], AF.Exp)
            ssum = pb.tile([128, 1], F32)
            nc.vector.tensor_reduce(ssum[:], attn_e[:], axis=mybir.AxisListType.X,
                                    op=mybir.AluOpType.add)
            rsum = pb.tile([128, 1], F32)
            nc.vector.reciprocal(rsum[:], ssum[:])
            attn_o = pb.tile([128, S], F32)
            nc.vector.tensor_scalar_mul(attn_o[:], attn_e[:], rsum[:])
            nc.sync.dma_start(attn_d[b], attn_o[:])

            # ---- context = attn @ value ----
            attnT = pb.tile([128, NK, 128], F32)                # (s%128, k, t)
            for k in range(NK):
                ps_t = ps_small.tile([128, 128], F32, tag="ps_prep")
                nc.tensor.transpose(ps_t[:], attn_o[:, ts(k, 128)], ident[:])
                nc.vector.tensor_copy(attnT[:, k, :], ps_t[:])
            ps_c = ps_ctx.tile([128, D], F32)
            for k in range(NK):
                nc.tensor.matmul(
                    ps_c[:], attnT[:, k, :], v_nats[b][:, k, :],
                    start=(k == 0), stop=(k == NK - 1),
                )
            ctx_sb = pb.tile([128, D], F32)
            nc.vector.tensor_copy(ctx_sb[:], ps_c[:])
            nc.sync.dma_start(ctx_d[b], ctx_sb[:])

    nc.compile()
    return nc


_NC_CACHE = None


def _get_nc():
    global _NC_CACHE
    if _NC_CACHE is None:
        _NC_CACHE = build_bass()
    return _NC_CACHE


def _shard_inputs(query, value, mask, W1, W2, scale):
    w1_r = np.ascontiguousarray(
        np.asarray(W1, dtype=np.float32).reshape(ND, 128, U).transpose(1, 0, 2))
    w2_r = np.ascontiguousarray(
        np.asarray(W2, dtype=np.float32).reshape(ND, 128, U).transpose(1, 0, 2))
    scale_r = np.ascontiguousarray(np.broadcast_to(
        np.asarray(scale, dtype=np.float32).reshape(NJ, 128).T.astype(np.float16)[:, :, None],
        (128, NJ, 32)))
    in_maps = []
    for c in range(NCORES):
        sl = slice(c * BPC, (c + 1) * BPC)
        in_maps.append({
            "query": np.ascontiguousarray(np.asarray(query[sl], dtype=np.float32)),
            "value": np.ascontiguousarray(np.asarray(value[sl], dtype=np.float32)),
            "mask": np.ascontiguousarray(
                np.asarray(mask[sl]).astype(np.uint8).reshape(1, BPC, S)),
            "W1": w1_r,
            "W2": w2_r,
            "scale": scale_r,
        })
    return in_maps


def run(query, value, mask, W1, W2, scale, **run_kwargs):
    nc = _get_nc()
    in_maps = _shard_inputs(query, value, mask, W1, W2, scale)
    res = run_bass_kernel_spmd(nc, in_maps, core_ids=list(range(NCORES)), **run_kwargs)
    context = np.concatenate([r["context"] for r in res.results], axis=0)
    attn = np.concatenate([r["attn"] for r in res.results], axis=0)
    return (context, attn), res


def kernel(query, value, mask, W1, W2, scale):
    (context, attn), _ = run(query, value, mask, W1, W2, scale)
    return context, attn
